# revision 15
# baseline (speedup 1.0000x reference)
"""Trainium2 Bass kernel for nn_CrossAttention_31078383354530.

Reference (b=2, n=m=2048, qd=1024, cd=768, heads=8, dh=128):
    q = x @ Wq; k = ctx @ Wk; v = ctx @ Wv  (8 heads of 128)
    sim over the FLATTENED (b*n)=4096 token axis (batches mix - reference
    replicates an einops bug); softmax((sim-mean)*1.5+mean) ==
    softmax(1.5*scale*sim) exactly; out = attn @ v; y = out @ Wout + bout.

Sharding (8 cores): HEAD-sharded tensor parallel. Every core gets the FULL
x^T / ctx^T plus only its head's Wq/Wk/Wv columns and Wout rows. Each core
computes its head's whole 4096x4096 attention locally (no K/V exchange at
all), projects its head's contribution to y^T per 512-token q-chunk, and a
per-chunk bf16 ReduceScatter(add) over qd-chunks sums the 8 heads while the
next chunk computes. Core c ends up with y^T rows [128c,128c+128) for all
4096 tokens; the host reassembles. bout/8 is folded into each partial.

Why this beats the 340us AllGather baseline:
  - microbench: the PE sustains 216ns per 512-col bf16 matmul (full 2.4GHz)
    for 100us+ under an 8-core storm, and keeps that p-state through short
    stalls. The old kernel's K/V AllGather chain (49us barrier + 8x20us)
    paced attention with long stalls that kept the PE at ~1.2GHz.
  - head sharding has identical per-core FLOPs but only exchanges the tiny
    1MB-per-chunk output partials, pipelined behind compute.
  - attention is paced by scalar-engine exp (~15.5us per 512-q chunk); the
    PE stream (sim groups + 4-group-lagged pv + out-proj partials) is
    emitted so it never waits on exp.
  - V is produced as V^T (512-col matmuls) then flipped to [ctx,dh] tiles
    by 32 DMA xbar transposes (no PE/DVE cost).
  - softmax denominators: DVE pairwise tree -> ones-matmul column sum ->
    reciprocal_approx_fast -> ones-broadcast matmul -> DVE multiply.
"""

import sys

if "/opt/trn_rl_repo" not in sys.path:
    sys.path.insert(0, "/opt/trn_rl_repo")

import ml_dtypes
import numpy as np

import concourse.bass as bass  # noqa: F401
import concourse.mybir as mybir
import concourse.tile as tile
from concourse import bacc, bass_utils

F32 = mybir.dt.float32
BF16 = mybir.dt.bfloat16
AF = mybir.ActivationFunctionType
ALU = mybir.AluOpType

P = 128
N_CORES = 8
HEADS = 8
DH = 128
TOK = 4096              # b*n flattened token axis
QD = 1024
CD = 768
KC = QD // P            # 8 qd chunks
CC = CD // P            # 6 cd chunks
JT = TOK // P           # 32 ctx j-tiles
QC = 8                  # q-chunks per core
QW = TOK // QC          # 512 q tokens per chunk
NG = 11                 # sim groups per chunk (3,3,...,3,2)
LAG = 4                 # pv trails sim by LAG positions
TAU_SCALE = 1.5 * (DH ** -0.5)

_CACHE = {}


def _build():
    nc = bacc.Bacc(num_devices=N_CORES)

    xT = nc.declare_dram_parameter("xT", [QD, TOK], BF16, isOutput=False)
    cT = nc.declare_dram_parameter("cT", [CD, TOK], BF16, isOutput=False)
    wq = nc.declare_dram_parameter("wq", [P, KC, DH], BF16, isOutput=False)
    wk = nc.declare_dram_parameter("wk", [P, CC, DH], BF16, isOutput=False)
    wv = nc.declare_dram_parameter("wv", [P, CC, DH], BF16, isOutput=False)
    wo = nc.declare_dram_parameter("wo", [P, QD], BF16, isOutput=False)
    ones16 = nc.declare_dram_parameter("ones16", [P, P], BF16, isOutput=False)
    ones32 = nc.declare_dram_parameter("ones32", [P, P], F32, isOutput=False)
    yT = nc.declare_dram_parameter("yT", [QC, P, QW], BF16, isOutput=True)

    with tile.TileContext(nc) as tc:
        with (
            tc.tile_pool(name="const", bufs=1) as const,
            tc.tile_pool(name="sb", bufs=1) as sb,
            tc.tile_pool(name="ps", bufs=1, space="PSUM") as ps,
            tc.tile_pool(name="dram", bufs=1, space="DRAM") as dram,
        ):
            # ---------------- DRAM exchange buffers ----------------
            rs_in = [dram.tile([N_CORES, P, QW], BF16, name=f"rs_in{q}")
                     for q in range(QC)]
            rs_out = [dram.tile([P, QW], BF16, name=f"rs_out{q}")
                      for q in range(QC)]

            # ---------------- input loads (K/V operands first) -----
            cts = []
            for k in range(CC):
                t = sb.tile([P, TOK], BF16, name=f"cts{k}", tag="cts", bufs=CC)
                nc.sync.dma_start(t[:], cT[k * P:(k + 1) * P, :])
                cts.append(t)
            wk_sb = const.tile([P, CC, DH], BF16, name="wk_sb")
            nc.sync.dma_start(wk_sb[:], wk[:, :, :])
            wv_sb = const.tile([P, CC, DH], BF16, name="wv_sb")
            nc.sync.dma_start(wv_sb[:], wv[:, :, :])
            xts = []
            for k in range(KC):
                t = sb.tile([P, TOK], BF16, name=f"xts{k}", tag="xts", bufs=KC)
                nc.sync.dma_start(t[:], xT[k * P:(k + 1) * P, :])
                xts.append(t)
            wq_sb = const.tile([P, KC, DH], BF16, name="wq_sb")
            nc.sync.dma_start(wq_sb[:], wq[:, :, :])
            wo_sb = const.tile([P, QD], BF16, name="wo_sb")
            nc.sync.dma_start(wo_sb[:], wo[:, :])
            o16_sb = const.tile([P, P], BF16, name="o16_sb")
            nc.sync.dma_start(o16_sb[:], ones16[:, :])
            o32_sb = const.tile([P, P], F32, name="o32_sb")
            nc.sync.dma_start(o32_sb[:], ones32[:, :])

            # ---------------- persistent SBUF results --------------
            kh = [sb.tile([P, QW], BF16, name=f"kh{i}", tag="kh", bufs=8)
                  for i in range(8)]      # K^T  [dh, ctx] in 512-chunks
            vTs = [sb.tile([P, QW], BF16, name=f"vT{i}", tag="vT", bufs=8)
                   for i in range(8)]     # V^T  [dh, ctx]
            vsb = [sb.tile([P, QW], BF16, name=f"vs{i}", tag="vs", bufs=8)
                   for i in range(8)]     # V    [ctx, dh] 4 j-tiles per tile
            qsb = [sb.tile([P, QW], BF16, name=f"qs{i}", tag="qs", bufs=8)
                   for i in range(8)]     # Q^T  [dh, q]

            pj_ctr = [0]

            def proj_tile(dst, w_sb, mov_tiles, nk, chunks, mov_of):
                """One [128,1536] psum tile holding len(chunks) 512-wide
                accumulations (contract over nk 128-chunks); copy each to
                dst[chunk] on DVE."""
                pj_ctr[0] += 1
                pt = ps.tile([P, 3 * QW], F32, name=f"pj{pj_ctr[0]}",
                             tag="sim", bufs=2)
                for gi, c in enumerate(chunks):
                    for k in range(nk):
                        nc.tensor.matmul(pt[:, gi * QW:(gi + 1) * QW],
                                         w_sb[:, k],
                                         mov_of(mov_tiles, k, c),
                                         start=(k == 0), stop=(k == nk - 1))
                for gi, c in enumerate(chunks):
                    nc.vector.tensor_copy(dst[c][:], pt[:, gi * QW:(gi + 1) * QW])

            mov_ctx = lambda tiles, k, c: tiles[k][:, c * QW:(c + 1) * QW]

            # K proj (all of it, first) + Q chunk 0-2
            proj_tile(kh, wk_sb, cts, CC, [0, 1, 2], mov_ctx)
            proj_tile(qsb, wq_sb, xts, KC, [0, 1, 2], mov_ctx)
            proj_tile(kh, wk_sb, cts, CC, [3, 4, 5], mov_ctx)
            proj_tile(kh, wk_sb, cts, CC, [6, 7], mov_ctx)
            # V^T chunks 0-2 early so vsb j0..11 exist for pv of chunk 0
            proj_tile(vTs, wv_sb, cts, CC, [0, 1, 2], mov_ctx)
            for j in range(12):
                nc.sync.dma_start(
                    vsb[j // 4][:, (j % 4) * DH:(j % 4 + 1) * DH],
                    vTs[j // 4][:, (j % 4) * DH:(j % 4 + 1) * DH],
                    transpose=True)

            # remaining projection work, doled out one item per position
            fillers = []
            fillers.append(lambda: proj_tile(vTs, wv_sb, cts, CC, [3, 4, 5],
                                             mov_ctx))
            def _vt_flip(j0):
                def f():
                    for j in range(j0, j0 + 4):
                        nc.sync.dma_start(
                            vsb[j // 4][:, (j % 4) * DH:(j % 4 + 1) * DH],
                            vTs[j // 4][:, (j % 4) * DH:(j % 4 + 1) * DH],
                            transpose=True)
                return f
            fillers.append(_vt_flip(12))
            fillers.append(_vt_flip(16))
            fillers.append(lambda: proj_tile(vTs, wv_sb, cts, CC, [6, 7],
                                             mov_ctx))
            fillers.append(_vt_flip(20))
            fillers.append(_vt_flip(24))
            fillers.append(_vt_flip(28))
            fillers.append(lambda: proj_tile(qsb, wq_sb, xts, KC, [3, 4, 5],
                                             mov_ctx))
            fillers.append(lambda: proj_tile(qsb, wq_sb, xts, KC, [6, 7],
                                             mov_ctx))

            # ---------------- attention position stream ------------
            # position u = qc*NG + g ; at u: sim(u), filler, pv(u-LAG),
            # chunk-qc extras at fixed offsets past the chunk's last sim.
            groups = [list(range(3 * g, min(3 * g + 3, JT)))
                      for g in range(NG)]

            sim_ctx = {}   # u -> dict for pending pv
            at_of = {}     # u -> at tile
            chunk = {}     # qc -> state dict

            def emit_sim(u):
                qc, g = divmod(u, NG)
                js = groups[g]
                w = len(js) * QW
                sim_ps = ps.tile([P, 3 * QW], F32, name=f"s{u}", tag="sim",
                                 bufs=2)
                for jj, j in enumerate(js):
                    nc.tensor.matmul(
                        sim_ps[:, jj * QW:(jj + 1) * QW],
                        kh[j // 4][:, (j % 4) * DH:(j % 4 + 1) * DH],
                        qsb[qc][:],
                        start=True, stop=True)
                at = sb.tile([P, 3 * QW], BF16, name=f"at{u}", tag="at",
                             bufs=LAG + 2)
                nc.scalar.activation(at[:, :w], sim_ps[:, :w], AF.Exp,
                                     scale=TAU_SCALE)
                at_of[u] = at
                # DVE tree: join pairs as they become available
                st = chunk.setdefault(qc, {"lvl": [], "pend": None})
                if g < 10:
                    if st["pend"] is None:
                        st["pend"] = at
                    else:
                        tr = sb.tile([P, 3 * QW], BF16, name=f"tr{u}",
                                     tag="tr", bufs=4)
                        nc.vector.tensor_tensor(tr[:], st["pend"][:], at[:],
                                                ALU.add)
                        st["pend"] = None
                        st["lvl"].append(tr)
                        while len(st["lvl"]) >= 2:
                            a = st["lvl"].pop(0)
                            b = st["lvl"].pop(0)
                            tr2 = sb.tile([P, 3 * QW], BF16, name=f"tr{u}b",
                                          tag="tr", bufs=4)
                            nc.vector.tensor_tensor(tr2[:], a[:], b[:],
                                                    ALU.add)
                            st["lvl"].append(tr2)

            def emit_pv(u):
                qc, g = divmod(u, NG)
                js = groups[g]
                st = chunk[qc]
                if g == 0:
                    st["pv"] = ps.tile([P, QW], F32, name=f"pv{qc}", tag="pv",
                                       bufs=2)
                at = at_of.pop(u)
                for jj, j in enumerate(js):
                    nc.tensor.matmul(st["pv"][:],
                                     vsb[j // 4][:, (j % 4) * DH:(j % 4 + 1) * DH],
                                     at[:, jj * QW:(jj + 1) * QW],
                                     start=(j == 0), stop=(j == JT - 1))

            def emit_colsum(qc):
                # fold tree root + last(2-wide) group -> rs_sum, then ones
                # matmul -> [1,512] column sums
                st = chunk[qc]
                root = st["lvl"].pop()
                assert not st["lvl"] and st["pend"] is None
                last = at_of[qc * NG + 10]  # pv pops it one position later
                f1 = sb.tile([P, QW], BF16, name=f"f1_{qc}", tag="f1", bufs=2)
                nc.vector.tensor_tensor(f1[:], root[:, :QW],
                                        root[:, QW:2 * QW], ALU.add)
                f2 = sb.tile([P, QW], BF16, name=f"f2_{qc}", tag="f2", bufs=2)
                nc.vector.tensor_tensor(f2[:], f1[:], root[:, 2 * QW:3 * QW],
                                        ALU.add)
                f3 = sb.tile([P, QW], BF16, name=f"f3_{qc}", tag="f3", bufs=2)
                nc.vector.tensor_tensor(f3[:], f2[:], last[:, :QW], ALU.add)
                rs_sum = sb.tile([P, QW], BF16, name=f"rs{qc}", tag="rss",
                                 bufs=2)
                nc.vector.tensor_tensor(rs_sum[:], f3[:], last[:, QW:2 * QW],
                                        ALU.add)
                cs = ps.tile([P, QW], F32, name=f"cs{qc}", tag="pv", bufs=2)
                nc.tensor.matmul(cs[:1, :], o16_sb[:, :1], rs_sum[:],
                                 start=True, stop=True)
                st["cs"] = cs
                recip = sb.tile([P, QW], F32, name=f"rcp{qc}", tag="rcp",
                                bufs=2)
                nc.vector.reciprocal_approx_fast(recip[:1, :], cs[:1, :])
                st["recip"] = recip

            def emit_bcast(qc):
                st = chunk[qc]
                den = ps.tile([P, 3 * QW], F32, name=f"den{qc}", tag="sim",
                              bufs=2)
                nc.tensor.matmul(den[:, :QW], o32_sb[:1, :],
                                 st["recip"][:1, :], start=True, stop=True)
                den_sb = sb.tile([P, QW], F32, name=f"dsb{qc}", tag="dsb",
                                 bufs=2)
                nc.vector.tensor_copy(den_sb[:], den[:, :QW])
                osb = sb.tile([P, QW], BF16, name=f"osb{qc}", tag="osb",
                              bufs=2)
                nc.vector.tensor_tensor(osb[:], st["pv"][:], den_sb[:],
                                        ALU.mult)
                st["osb"] = osb

            def emit_proj(qc, ccs):
                # partial y^T chunks for this head: p[cc] = wo[:,cc]^T @ osb
                st = chunk[qc]
                pt = ps.tile([P, 3 * QW], F32, name=f"pp{qc}_{ccs[0]}",
                             tag="sim", bufs=2)
                for gi, cc2 in enumerate(ccs):
                    nc.tensor.matmul(pt[:, gi * QW:(gi + 1) * QW],
                                     wo_sb[:, cc2 * DH:(cc2 + 1) * DH],
                                     st["osb"][:], start=True, stop=True)
                for gi, cc2 in enumerate(ccs):
                    pc = sb.tile([P, QW], BF16, name=f"pc{qc}_{cc2}",
                                 tag="pc", bufs=4)
                    nc.vector.tensor_copy(pc[:], pt[:, gi * QW:(gi + 1) * QW])
                    nc.sync.dma_start(rs_in[qc][cc2], pc[:])

            def emit_rs(qc):
                nc.gpsimd.collective_compute(
                    "ReduceScatter", ALU.add,
                    replica_groups=[list(range(N_CORES))],
                    ins=[rs_in[qc].opt()], outs=[rs_out[qc].opt()])

            NPOS = QC * NG
            extras = {}  # position -> list of thunks
            for qc in range(QC):
                end = qc * NG + (NG - 1)   # position of last sim of chunk
                extras.setdefault(end + 3, []).append(
                    lambda q=qc: emit_colsum(q))
                extras.setdefault(end + 4, []).append(
                    lambda q=qc: emit_bcast(q))
                extras.setdefault(end + 5, []).append(
                    lambda q=qc: emit_proj(q, [0, 1, 2]))
                extras.setdefault(end + 6, []).append(
                    lambda q=qc: emit_proj(q, [3, 4, 5]))
                extras.setdefault(end + 7, []).append(
                    lambda q=qc: emit_proj(q, [6, 7]))
                extras.setdefault(end + 8, []).append(
                    lambda q=qc: emit_rs(q))

            fill_iter = iter(fillers)
            for u in range(NPOS + LAG + 11):
                if u < NPOS:
                    emit_sim(u)
                    nxt = next(fill_iter, None)
                    if nxt is not None:
                        nxt()
                if u - LAG >= 0 and u - LAG < NPOS:
                    emit_pv(u - LAG)
                for th in extras.get(u, []):
                    th()

            # ---------------- final output DMAs --------------------
            for qc in range(QC):
                nc.sync.dma_start(yT.ap()[qc], rs_out[qc][:, :])

    nc.compile()
    return nc


def _get_nc():
    if "nc" not in _CACHE:
        _CACHE["nc"] = _build()
    return _CACHE["nc"]


def _bf16(a):
    return np.ascontiguousarray(
        np.asarray(a, np.float32).astype(ml_dtypes.bfloat16))


def _prep_in_maps(x, context, Wq, Wk, Wv, Wout, bout):
    xT = _bf16(np.asarray(x, np.float32).reshape(TOK, QD).T)
    cT = _bf16(np.asarray(context, np.float32).reshape(TOK, CD).T)
    Wq = np.asarray(Wq, np.float32)
    Wk = np.asarray(Wk, np.float32)
    Wv = np.asarray(Wv, np.float32)
    Wout = np.asarray(Wout, np.float32)
    o16 = np.ones((P, P), np.float32)
    in_maps = []
    for c in range(N_CORES):
        h = slice(c * DH, (c + 1) * DH)
        in_maps.append({
            "xT": xT, "cT": cT,
            "wq": _bf16(Wq[:, h].reshape(KC, P, DH).transpose(1, 0, 2)),
            "wk": _bf16(Wk[:, h].reshape(CC, P, DH).transpose(1, 0, 2)),
            "wv": _bf16(Wv[:, h].reshape(CC, P, DH).transpose(1, 0, 2)),
            "wo": _bf16(Wout[h, :]),
            "ones16": _bf16(o16),
            "ones32": np.ascontiguousarray(o16),
        })
    return in_maps


def _assemble(results, bout):
    y = np.empty((TOK, QD), dtype=np.float32)
    for c in range(N_CORES):
        yt = np.asarray(results[c]["yT"], dtype=np.float32)  # [QC, P, QW]
        for qc in range(QC):
            y[qc * QW:(qc + 1) * QW, c * P:(c + 1) * P] = yt[qc].T
    y += np.asarray(bout, np.float32)[None, :]
    return y.reshape(2, TOK // 2, QD)


def run(inputs, trace=False, **kw):
    nc = _get_nc()
    in_maps = _prep_in_maps(**inputs)
    res = bass_utils.run_bass_kernel_spmd(
        nc, in_maps, core_ids=list(range(N_CORES)), trace=trace, **kw)
    return _assemble(res.results, inputs["bout"]), res


def kernel(**inputs):
    out, _ = run(inputs, trace=False)
    return out
